# revision 2
# baseline (speedup 1.0000x reference)
"""Cross-WindowAttention Trainium2 kernel.

Full inputs -> shard batch dim over 8 NeuronCores -> bass/Tile kernel per core
-> gather. Host-side numpy does layout prep (transposes to feature-major,
bf16 conversion, combined rpb+mask bias table); the Bass kernel does all
matmul/softmax compute.

Per-core pipeline (shard = 256 windows of 64 tokens, 16384 rows):
 - qkv projections on PE in bf16, contraction over concat(x,y) for k/v.
   q,k produced feature-major [feat, rows]; v row-major per window [64, 512].
 - attention per (head-pair, 8-window chunk) in one [128, 512] PSUM bank:
   bias added via identity-matmul accumulation, exp on ScalarE (scale folded),
   softmax denominator via VectorE free-axis reduce + reciprocal + bcast mul,
   PE-transpose of normalized weights, PV matmul with v stationary.
 - output projection with attention-output tiles stationary -> row-major
   result, biases via ones-row matmul, contiguous DMA out.

The chunk loop is software-pipelined by one chunk: the small attention/proj
matmul groups of chunk c-1 are emitted interleaved between the large qkv
matmul groups of chunk c, keeping the PE array duty cycle high enough that
the HAM activity monitor does not clock-gate it to half speed.
"""
import numpy as np
import ml_dtypes

import concourse.bacc as bacc
import concourse.mybir as mybir
from concourse.tile import TileContext
from concourse.bass_utils import run_bass_kernel_spmd

F32 = mybir.dt.float32
BF16 = mybir.dt.bfloat16
BF = ml_dtypes.bfloat16

N_CORES = 8
B_FULL = 2048
N = 64                      # window size (tokens per window)
C = 512                     # channels
H = 16                      # heads
HD = 32                     # head dim
CX = 512                    # x feature dim
CY = 1000                   # y feature dim
CYP = 1024                  # y feature dim padded to multiple of 128
SCALE = HD ** -0.5

B_SHARD = B_FULL // N_CORES             # 256 windows per core
WIN_PER_CHUNK = 8
ROWS_PER_CHUNK = WIN_PER_CHUNK * N      # 512
N_CHUNKS = B_SHARD // WIN_PER_CHUNK     # 32

KT_X = CX // 128            # 4 contraction tiles from x
KT_Y = CYP // 128           # 8 contraction tiles from y (padded)
FT_Q = C // 128             # 4 feature tiles per projection output


def build_nc(n_chunks=N_CHUNKS):
    rows = n_chunks * ROWS_PER_CHUNK
    nc = bacc.Bacc("TRN2", target_bir_lowering=False)

    xt = nc.dram_tensor("xt", [CX, rows], BF16, kind="ExternalInput")
    yt = nc.dram_tensor("yt", [CYP, rows], BF16, kind="ExternalInput")
    w1 = nc.dram_tensor("w1", [CX, 3 * C], BF16, kind="ExternalInput")
    w2 = nc.dram_tensor("w2", [CYP, 3 * C], BF16, kind="ExternalInput")
    wp = nc.dram_tensor("wp", [4, 128, C], BF16, kind="ExternalInput")  # quad-permuted rows
    cb = nc.dram_tensor("cb", [8, 8, 128, 512], BF16, kind="ExternalInput")
    bq = nc.dram_tensor("bq", [128, FT_Q], F32, kind="ExternalInput")
    bp = nc.dram_tensor("bp", [128, C], F32, kind="ExternalInput")
    ident = nc.dram_tensor("ident", [128, 128], BF16, kind="ExternalInput")
    out = nc.dram_tensor("out", [rows, C], F32, kind="ExternalOutput")

    with TileContext(nc) as tc:
        with tc.tile_pool(name="const", bufs=1) as constp, \
             tc.tile_pool(name="wpool", bufs=1) as wpool, \
             tc.tile_pool(name="stream", bufs=2) as stream, \
             tc.tile_pool(name="acts", bufs=2) as acts, \
             tc.tile_pool(name="small", bufs=3) as small, \
             tc.tile_pool(name="pbig", bufs=2, space="PSUM") as pbig, \
             tc.tile_pool(name="pattn", bufs=2, space="PSUM") as pattn, \
             tc.tile_pool(name="ptnk", bufs=2, space="PSUM") as ptnk, \
             tc.tile_pool(name="pot", bufs=2, space="PSUM") as pot:

            # ---- resident constants / weights
            w1_sb = wpool.tile([128, KT_X, 3 * C], BF16)
            nc.sync.dma_start(out=w1_sb, in_=w1.rearrange("(a p) f -> p a f", p=128))
            w2_sb = wpool.tile([128, KT_Y, 3 * C], BF16)
            nc.sync.dma_start(out=w2_sb, in_=w2.rearrange("(a p) f -> p a f", p=128))
            wp_sb = wpool.tile([128, 4, C], BF16)
            nc.sync.dma_start(out=wp_sb, in_=wp.rearrange("a p f -> p a f"))
            bq_sb = constp.tile([128, FT_Q], F32)
            nc.sync.dma_start(out=bq_sb, in_=bq[:, :])
            bp_sb = constp.tile([128, C], F32)
            nc.sync.dma_start(out=bp_sb, in_=bp[:, :])
            id_sb = constp.tile([128, 128], BF16)
            nc.sync.dma_start(out=id_sb, in_=ident[:, :])

            xt_r = xt.rearrange("(a p) r -> p a r", p=128)
            yt_r = yt.rearrange("(a p) r -> p a r", p=128)

            st = {}  # per-chunk live tiles

            def emit_dma(c):
                r0 = c * ROWS_PER_CHUNK
                s = {}
                s["xt"] = stream.tile([128, KT_X, ROWS_PER_CHUNK], BF16, tag="xt", name="xt")
                nc.sync.dma_start(out=s["xt"], in_=xt_r[:, :, r0:r0 + ROWS_PER_CHUNK])
                s["yt"] = stream.tile([128, KT_Y, ROWS_PER_CHUNK], BF16, tag="yt", name="yt")
                nc.sync.dma_start(out=s["yt"], in_=yt_r[:, :, r0:r0 + ROWS_PER_CHUNK])
                s["cb"] = stream.tile([128, 8, 512], BF16, tag="cb", name="cbt")
                nc.sync.dma_start(out=s["cb"],
                                  in_=cb[c % 8].rearrange("hp p f -> p hp f"))
                s["q"] = acts.tile([128, FT_Q, ROWS_PER_CHUNK], BF16, tag="q", name="qsb")
                s["k"] = acts.tile([128, FT_Q, ROWS_PER_CHUNK], BF16, tag="k", name="ksb")
                s["v"] = acts.tile([64, WIN_PER_CHUNK, C], BF16, tag="v", name="vsb")
                s["ot"] = acts.tile([128, 4 * ROWS_PER_CHUNK], BF16, tag="ot", name="otsb")
                st[c] = s

            def emit_qkv_group(c, g):
                s = st[c]
                if g < FT_Q:                      # q projection, feature tile g
                    ft = g
                    bank = pbig.tile([128, ROWS_PER_CHUNK], F32, tag="pq")
                    for kt in range(KT_X):
                        nc.tensor.matmul(
                            bank[:, :],
                            w1_sb[:, kt, 128 * ft:128 * (ft + 1)],
                            s["xt"][:, kt, :],
                            start=(kt == 0), stop=(kt == KT_X - 1))
                    nc.scalar.activation(
                        s["q"][:, ft, :], bank[:, :],
                        mybir.ActivationFunctionType.Identity,
                        bias=bq_sb[:, ft:ft + 1])
                elif g < 2 * FT_Q:                # k projection, feature tile g-4
                    ft = g - FT_Q
                    bank = pbig.tile([128, ROWS_PER_CHUNK], F32, tag="pq")
                    for kt in range(KT_X):
                        nc.tensor.matmul(
                            bank[:, :],
                            w1_sb[:, kt, C + 128 * ft:C + 128 * (ft + 1)],
                            s["xt"][:, kt, :],
                            start=(kt == 0), stop=False)
                    for kt in range(KT_Y):
                        nc.tensor.matmul(
                            bank[:, :],
                            w2_sb[:, kt, C + 128 * ft:C + 128 * (ft + 1)],
                            s["yt"][:, kt, :],
                            start=False, stop=(kt == KT_Y - 1))
                    nc.scalar.copy(s["k"][:, ft, :], bank[:, :])
                else:                             # v projection, row tile g-8
                    rt = g - 2 * FT_Q
                    bank = pbig.tile([128, C], F32, tag="pq")
                    for kt in range(KT_X):
                        nc.tensor.matmul(
                            bank[:, :],
                            s["xt"][:, kt, 128 * rt:128 * (rt + 1)],
                            w1_sb[:, kt, 2 * C:3 * C],
                            start=(kt == 0), stop=False)
                    for kt in range(KT_Y):
                        nc.tensor.matmul(
                            bank[:, :],
                            s["yt"][:, kt, 128 * rt:128 * (rt + 1)],
                            w2_sb[:, kt, 2 * C:3 * C],
                            start=False, stop=(kt == KT_Y - 1))
                    nc.scalar.copy(s["v"][:, 2 * rt, :], bank[0:64, :])
                    nc.vector.tensor_copy(s["v"][:, 2 * rt + 1, :], bank[64:128, :])

            def emit_attn_group(c, hp):
                s = st[c]
                bank = pattn.tile([128, 512], F32, tag="pattn")
                for sw in range(WIN_PER_CHUNK):
                    for hh in range(2):
                        h = 2 * hp + hh
                        pq = 32 * (h % 4)
                        ft = h // 4
                        nc.tensor.matmul(
                            bank[64 * hh:64 * (hh + 1), 64 * sw:64 * (sw + 1)],
                            s["q"][pq:pq + 32, ft, 64 * sw:64 * (sw + 1)],
                            s["k"][pq:pq + 32, ft, 64 * sw:64 * (sw + 1)],
                            start=True, stop=True, skip_group_check=True,
                            tile_position=(pq, 64 * hh))
                # combined rpb+mask bias (pre-divided by SCALE) added on DVE
                nc.vector.tensor_tensor(out=bank[:, :], in0=bank[:, :],
                                        in1=s["cb"][:, hp, :],
                                        op=mybir.AluOpType.add)
                expa = small.tile([128, 8, 64], BF16, tag="expa")
                nc.scalar.activation(
                    expa.rearrange("p s m -> p (s m)"), bank[:, :],
                    mybir.ActivationFunctionType.Exp, scale=SCALE)
                den = small.tile([128, 8], F32, tag="den")
                nc.vector.tensor_reduce(
                    den[:, :], expa[:, :, :],
                    axis=mybir.AxisListType.X, op=mybir.AluOpType.add)
                rden = small.tile([128, 8], F32, tag="rden")
                nc.vector.reciprocal(rden[:, :], den[:, :])
                norma = small.tile([128, 8, 64], BF16, tag="norma")
                nc.vector.tensor_tensor(
                    out=norma[:, :, :], in0=expa[:, :, :],
                    in1=rden.unsqueeze(-1).broadcast_to([128, 8, 64]),
                    op=mybir.AluOpType.mult)
                # transpose normalized weights: [(2h,n), m] -> [m, (2h,n)]
                tnk_sb = small.tile([64, 8, 128], BF16, tag="tnk")
                for half in range(2):
                    tbank = ptnk.tile([64, 512], BF16, tag="ptnk")
                    for j in range(4):
                        sw = 4 * half + j
                        nc.tensor.transpose(
                            tbank[:, 128 * j:128 * (j + 1)],
                            norma[:, sw, :], id_sb[:, :])
                    dst = tnk_sb[:, 4 * half:4 * half + 4, :] \
                        .rearrange("p s f -> p (s f)")
                    if half == 0:
                        nc.scalar.copy(dst, tbank[:, :])
                    else:
                        nc.vector.tensor_copy(dst, tbank[:, :])
                # PV: v stationary, transposed attn moving
                obank = pot.tile([64, 512], F32, tag="pot")
                for sw in range(WIN_PER_CHUNK):
                    for hh in range(2):
                        h = 2 * hp + hh
                        nc.tensor.matmul(
                            obank[32 * hh:32 * (hh + 1), 64 * sw:64 * (sw + 1)],
                            s["v"][:, sw, HD * h:HD * (h + 1)],
                            tnk_sb[:, sw, 64 * hh:64 * (hh + 1)],
                            start=True, stop=True)
                # stage to SBUF: partition 32*(h%4)+d, free (t, q=h//4, w, m)
                nc.scalar.copy(
                    s["ot"][64 * (hp % 2):64 * (hp % 2) + 64, :]
                    .rearrange("p (t q w m) -> p t q w m", t=4, q=4, w=2)
                    [:, :, hp // 2, :, :],
                    obank.rearrange("p (t w m) -> p t w m", t=4, w=2))

            def emit_proj_group(c, rt):
                s = st[c]
                r0 = c * ROWS_PER_CHUNK
                bank = pbig.tile([128, C], F32, tag="pq")
                for quad in range(4):
                    nc.tensor.matmul(
                        bank[:, :],
                        s["ot"].rearrange("p (t q f) -> p t q f", t=4, q=4)
                        [:, rt, quad, :],
                        wp_sb[:, quad, :],
                        start=(quad == 0), stop=(quad == 3))
                out_f32 = small.tile([128, C], F32, tag="outf")
                nc.vector.tensor_tensor(out=out_f32[:, :], in0=bank[:, :],
                                        in1=bp_sb[:, :], op=mybir.AluOpType.add)
                nc.sync.dma_start(
                    out=out[r0 + 128 * rt:r0 + 128 * (rt + 1), :],
                    in_=out_f32[:, :])

            # software pipeline: big qkv groups of chunk c interleaved with
            # small attention/proj groups of chunk c-1
            for c in range(n_chunks + 1):
                if c < n_chunks:
                    emit_dma(c)
                big = [("qkv", c, g) for g in range(12)] if c < n_chunks else []
                smalls = ([("attn", c - 1, hp) for hp in range(8)]
                          + [("proj", c - 1, rt) for rt in range(4)]) if c > 0 else []
                order = []
                for i in range(max(len(big), len(smalls))):
                    if i < len(big):
                        order.append(big[i])
                    if i < len(smalls):
                        order.append(smalls[i])
                for kind, cc, idx in order:
                    if kind == "qkv":
                        emit_qkv_group(cc, idx)
                    elif kind == "attn":
                        emit_attn_group(cc, idx)
                    else:
                        emit_proj_group(cc, idx)
                if c > 0:
                    del st[c - 1]
    nc.compile()
    return nc


_NC_CACHE = {}


def _get_nc(n_chunks):
    if n_chunks not in _NC_CACHE:
        _NC_CACHE[n_chunks] = build_nc(n_chunks)
    return _NC_CACHE[n_chunks]


def prep_shared(w_qkv1, b_qkv1, w_qkv2, b_qkv2, bias_table, rel_index, w_proj,
                b_proj, mask):
    """Host-side prep of weights/bias tables shared by all cores."""
    w1 = w_qkv1.astype(BF)
    w2 = np.zeros((CYP, 3 * C), np.float32)
    w2[:CY] = w_qkv2
    # k/v biases ride an all-ones row in the padded region of yT
    w2[CY, C:2 * C] = b_qkv1[C:2 * C] + b_qkv2[C:2 * C]
    w2[CY, 2 * C:] = b_qkv1[2 * C:] + b_qkv2[2 * C:]
    w2 = w2.astype(BF)
    # quad-permuted rows: wp[Q, p, :] = w_proj[32*(4Q + p//32) + p%32, :]
    wp = np.empty((4, 128, C), np.float32)
    for q in range(4):
        for g in range(4):
            wp[q, 32 * g:32 * (g + 1), :] = \
                w_proj[32 * (4 * q + g):32 * (4 * q + g) + 32, :]
    wp = wp.astype(BF)

    bq = b_qkv1[0:C].reshape(FT_Q, 128).T.astype(np.float32).copy()
    bp = np.broadcast_to(b_proj.astype(np.float32), (128, C)).copy()

    rpb = bias_table[rel_index.reshape(-1)].reshape(N, N, H).transpose(2, 0, 1)
    cbt = (rpb[None] + mask[:, None]) / SCALE          # [w, h, n, m]
    cb6 = cbt.reshape(8, 8, 8, 2, N, N)                # [c8, s, hp, hh, n, m]
    cbd = np.ascontiguousarray(cb6.transpose(0, 2, 3, 4, 1, 5)) \
        .reshape(8, 8, 128, 512).astype(BF)

    ident = np.eye(128, dtype=BF)
    return dict(w1=w1, w2=w2, wp=wp, bq=bq, bp=bp, cb=cbd, ident=ident)


def prep_core_inputs(x, y, shared, n_cores=N_CORES):
    """Split x, y along batch, transpose to feature-major, bf16."""
    B_, n, _ = x.shape
    rows = (B_ // n_cores) * n
    in_maps = []
    for i in range(n_cores):
        lo = i * (B_ // n_cores)
        hi = lo + B_ // n_cores
        xs = x[lo:hi].reshape(rows, CX)
        ys = y[lo:hi].reshape(rows, CY)
        xtb = np.ascontiguousarray(xs.T).astype(BF)
        ytb = np.zeros((CYP, rows), BF)
        ytb[:CY] = np.ascontiguousarray(ys.T).astype(BF)
        ytb[CY] = 1.0
        in_maps.append(dict(xt=xtb, yt=ytb, **shared))
    return in_maps


def kernel(x, y, mask, w_qkv1, b_qkv1, w_qkv2, b_qkv2, bias_table, rel_index,
           w_proj, b_proj, _n_cores=N_CORES, _trace=False):
    B_, n, _ = x.shape
    n_chunks = (B_ // _n_cores) // WIN_PER_CHUNK
    shared = prep_shared(np.asarray(w_qkv1), np.asarray(b_qkv1),
                         np.asarray(w_qkv2), np.asarray(b_qkv2),
                         np.asarray(bias_table), np.asarray(rel_index),
                         np.asarray(w_proj), np.asarray(b_proj),
                         np.asarray(mask))
    in_maps = prep_core_inputs(np.asarray(x), np.asarray(y), shared, _n_cores)
    nc = _get_nc(n_chunks)
    res = run_bass_kernel_spmd(nc, in_maps, core_ids=list(range(_n_cores)),
                               trace=_trace)
    outs = [res.results[i]["out"].reshape(B_ // _n_cores, n, C)
            for i in range(_n_cores)]
    full = np.concatenate(outs, axis=0)
    kernel.last_results = res
    kernel.last_ctx = (nc, in_maps)
    return full



# revision 5
# speedup vs baseline: 1.0791x; 1.0791x over previous
"""Cross-WindowAttention Trainium2 kernel (v3).

Full inputs -> shard batch dim over 8 NeuronCores -> bass/Tile kernel per core
-> gather. Host-side numpy does layout prep (chunk-major feature-major
transposes, fp8/bf16 conversion, combined rpb+mask bias table); the Bass
kernel does all matmul/softmax compute.

Per-core pipeline (shard = 256 windows of 64 tokens, 16384 rows):
 - q/k projections on PE in fp8e4 with DoubleRow perf mode (weights
   pre-scaled x16 on host, de-scaled at the PSUM->SBUF copy). v projection
   in bf16 (fp8 v fails the accuracy gate); v rows are duplicated into both
   partition halves (small DMAs) so PV matmuls can pack.
 - attention logits computed TRANSPOSED ([m, n]; k stationary) per
   (hq = 4 heads) x (8 windows) group into two PSUM banks, each holding a
   diagonal pair of PE tile positions; the two banks' matmuls overlap on
   the array (verified: same-column-group tiles are only legal across
   different PSUM banks).  Softmax denominator via a host-constant selector
   matmul that lands partition-replicated [128, 512]; normalization is
   fused into the PV->SBUF staging multiply on DVE.  No PE transposes.
 - output projection with attention-output tiles stationary in bf16,
   bias via DVE add -> row-major result, contiguous DMA out.

The chunk loop is software-pipelined by one chunk: attention/proj groups of
chunk c-1 are emitted interleaved between the qkv matmul groups of chunk c.
"""
import numpy as np
import ml_dtypes

import concourse.bacc as bacc
import concourse.mybir as mybir
from concourse.tile import TileContext
from concourse.bass_utils import run_bass_kernel_spmd

F32 = mybir.dt.float32
BF16 = mybir.dt.bfloat16
FP8 = mybir.dt.float8e4
BF = ml_dtypes.bfloat16
E4 = ml_dtypes.float8_e4m3

N_CORES = 8
B_FULL = 2048
N = 64                      # window size (tokens per window)
C = 512                     # channels
H = 16                      # heads
HD = 32                     # head dim
CX = 512                    # x feature dim
CY = 1000                   # y feature dim
CYP = 1024                  # y feature dim padded to multiple of 128
SCALE = HD ** -0.5
WS = 16.0                   # fp8 weight pre-scale
INV_WS = 1.0 / WS

B_SHARD = B_FULL // N_CORES             # 256 windows per core
WIN_PER_CHUNK = 8
ROWS_PER_CHUNK = WIN_PER_CHUNK * N      # 512
N_CHUNKS = B_SHARD // WIN_PER_CHUNK     # 32

KT_X = CX // 128            # 4 contraction tiles from x
KT_Y = CYP // 128            # 8 contraction tiles from y (padded)
FT_Q = C // 128             # 4 feature tiles per projection output

DR = mybir.MatmulPerfMode.DoubleRow


def build_nc(n_chunks=N_CHUNKS):
    rows = n_chunks * ROWS_PER_CHUNK
    nc = bacc.Bacc("TRN2", target_bir_lowering=False)

    xt8 = nc.dram_tensor("xt8", [n_chunks, 128, KT_X, 512], FP8, kind="ExternalInput")
    yt8 = nc.dram_tensor("yt8", [n_chunks, 128, KT_Y, 512], FP8, kind="ExternalInput")
    xb = nc.dram_tensor("xb", [n_chunks, 128, KT_X, 512], BF16, kind="ExternalInput")
    yb = nc.dram_tensor("yb", [n_chunks, 128, KT_Y, 512], BF16, kind="ExternalInput")
    w1qk = nc.dram_tensor("w1qk", [128, KT_X, 2 * C], FP8, kind="ExternalInput")
    w2k = nc.dram_tensor("w2k", [128, KT_Y, C], FP8, kind="ExternalInput")
    w1v = nc.dram_tensor("w1v", [128, KT_X, C], BF16, kind="ExternalInput")
    w2v = nc.dram_tensor("w2v", [128, KT_Y, C], BF16, kind="ExternalInput")
    wp = nc.dram_tensor("wp", [128, 4, C], BF16, kind="ExternalInput")
    cb = nc.dram_tensor("cb", [8, 128, 8, 512], BF16, kind="ExternalInput")
    bq = nc.dram_tensor("bq", [128, FT_Q], F32, kind="ExternalInput")
    bp = nc.dram_tensor("bp", [128, C], F32, kind="ExternalInput")
    sel = nc.dram_tensor("sel", [128, 64], BF16, kind="ExternalInput")
    out = nc.dram_tensor("out", [rows, C], F32, kind="ExternalOutput")

    with TileContext(nc) as tc:
        with tc.tile_pool(name="const", bufs=1) as constp, \
             tc.tile_pool(name="wpool", bufs=1) as wpool, \
             tc.tile_pool(name="stream", bufs=2) as stream, \
             tc.tile_pool(name="acts", bufs=2) as acts, \
             tc.tile_pool(name="epool", bufs=2) as epool, \
             tc.tile_pool(name="small", bufs=3) as small, \
             tc.tile_pool(name="pbig", bufs=2, space="PSUM") as pbig, \
             tc.tile_pool(name="ploga", bufs=2, space="PSUM") as ploga, \
             tc.tile_pool(name="plogb", bufs=2, space="PSUM") as plogb, \
             tc.tile_pool(name="pden", bufs=1, space="PSUM") as pden, \
             tc.tile_pool(name="pout", bufs=1, space="PSUM") as pout:

            # ---- resident constants / weights
            w1qk_sb = wpool.tile([128, KT_X, 2 * C], FP8)
            nc.sync.dma_start(out=w1qk_sb, in_=w1qk[:, :, :])
            w2k_sb = wpool.tile([128, KT_Y, C], FP8)
            nc.sync.dma_start(out=w2k_sb, in_=w2k[:, :, :])
            w1v_sb = wpool.tile([128, KT_X, C], BF16)
            nc.sync.dma_start(out=w1v_sb, in_=w1v[:, :, :])
            w2v_sb = wpool.tile([128, KT_Y, C], BF16)
            nc.sync.dma_start(out=w2v_sb, in_=w2v[:, :, :])
            wp_sb = wpool.tile([128, 4, C], BF16)
            nc.sync.dma_start(out=wp_sb, in_=wp[:, :, :])
            bq_sb = constp.tile([128, FT_Q], F32)
            nc.sync.dma_start(out=bq_sb, in_=bq[:, :])
            bp_sb = constp.tile([128, C], F32)
            nc.sync.dma_start(out=bp_sb, in_=bp[:, :])
            sel_sb = constp.tile([128, 64], BF16)
            nc.sync.dma_start(out=sel_sb, in_=sel[:, :])
            cb_sb = []
            for c8 in range(min(8, n_chunks)):
                t = constp.tile([128, 8, 512], BF16, name=f"cb{c8}")
                nc.sync.dma_start(out=t, in_=cb[c8])
                cb_sb.append(t)

            st = {}  # per-chunk live tiles

            def emit_dma(c):
                s = {}
                s["xt8"] = stream.tile([128, KT_X, 512], FP8, tag="xt8", name="xt8")
                nc.sync.dma_start(out=s["xt8"], in_=xt8[c])
                s["yt8"] = stream.tile([128, KT_Y, 512], FP8, tag="yt8", name="yt8")
                nc.sync.dma_start(out=s["yt8"], in_=yt8[c])
                s["xb"] = stream.tile([128, KT_X, 512], BF16, tag="xb", name="xb")
                nc.sync.dma_start(out=s["xb"], in_=xb[c])
                s["yb"] = stream.tile([128, KT_Y, 512], BF16, tag="yb", name="yb")
                nc.sync.dma_start(out=s["yb"], in_=yb[c])
                s["q"] = acts.tile([128, FT_Q, 512], BF16, tag="q", name="qsb")
                s["k"] = acts.tile([128, FT_Q, 512], BF16, tag="k", name="ksb")
                s["vd"] = acts.tile([128, 8, C], BF16, tag="vd", name="vdsb")
                s["ot"] = acts.tile([128, 4 * 512], BF16, tag="ot", name="otsb")
                st[c] = s

            def emit_qkv_group(c, g):
                s = st[c]
                if g < FT_Q:                      # q projection, feature tile g
                    ft = g
                    bank = pbig.tile([128, 512], F32, tag="pq")
                    for kp in range(KT_X // 2):
                        nc.tensor.matmul(
                            bank[:, :],
                            w1qk_sb[:, 2 * kp:2 * kp + 2, 128 * ft:128 * (ft + 1)],
                            s["xt8"][:, 2 * kp:2 * kp + 2, :],
                            start=(kp == 0), stop=(kp == KT_X // 2 - 1),
                            perf_mode=DR)
                    nc.scalar.activation(
                        s["q"][:, ft, :], bank[:, :],
                        mybir.ActivationFunctionType.Identity,
                        bias=bq_sb[:, ft:ft + 1], scale=INV_WS)
                elif g < 2 * FT_Q:                # k projection, feature tile g-4
                    ft = g - FT_Q
                    bank = pbig.tile([128, 512], F32, tag="pq")
                    for kp in range(KT_X // 2):
                        nc.tensor.matmul(
                            bank[:, :],
                            w1qk_sb[:, 2 * kp:2 * kp + 2,
                                    C + 128 * ft:C + 128 * (ft + 1)],
                            s["xt8"][:, 2 * kp:2 * kp + 2, :],
                            start=(kp == 0), stop=False, perf_mode=DR)
                    for kp in range(KT_Y // 2):
                        nc.tensor.matmul(
                            bank[:, :],
                            w2k_sb[:, 2 * kp:2 * kp + 2, 128 * ft:128 * (ft + 1)],
                            s["yt8"][:, 2 * kp:2 * kp + 2, :],
                            start=False, stop=(kp == KT_Y // 2 - 1), perf_mode=DR)
                    nc.vector.tensor_scalar_mul(s["k"][:, ft, :], bank[:, :], INV_WS)
                else:                             # v projection, row tile g-8
                    rt = g - 2 * FT_Q
                    bank = pbig.tile([128, C], F32, tag="pq")
                    for kt in range(KT_X):
                        nc.tensor.matmul(
                            bank[:, :],
                            s["xb"][:, kt, 128 * rt:128 * (rt + 1)],
                            w1v_sb[:, kt, :],
                            start=(kt == 0), stop=False)
                    for kt in range(KT_Y):
                        nc.tensor.matmul(
                            bank[:, :],
                            s["yb"][:, kt, 128 * rt:128 * (rt + 1)],
                            w2v_sb[:, kt, :],
                            start=False, stop=(kt == KT_Y - 1))
                    # windows (2rt, 2rt+1): duplicate rows into both halves
                    nc.vector.tensor_copy(s["vd"][0:64, 2 * rt, :], bank[0:64, :])
                    nc.vector.tensor_copy(s["vd"][64:128, 2 * rt + 1, :],
                                          bank[64:128, :])
                    nc.sync.dma_start(out=s["vd"][64:128, 2 * rt, :],
                                      in_=s["vd"][0:64, 2 * rt, :])
                    nc.sync.dma_start(out=s["vd"][0:64, 2 * rt + 1, :],
                                      in_=s["vd"][64:128, 2 * rt + 1, :])

            def emit_attn_group(c, hq):
                s = st[c]
                bankA = ploga.tile([128, 512], F32, tag="la")
                bankB = plogb.tile([128, 512], F32, tag="lb")
                # QK^T transposed: k stationary -> logits block [m, n]
                for wi in range(8):
                    for hi in range(4):
                        bank = bankA if hi < 2 else bankB
                        hj = hi % 2
                        nc.tensor.matmul(
                            bank[64 * hj:64 * (hj + 1), 64 * wi:64 * (wi + 1)],
                            s["k"][32 * hi:32 * hi + 32, hq, 64 * wi:64 * (wi + 1)],
                            s["q"][32 * hi:32 * hi + 32, hq, 64 * wi:64 * (wi + 1)],
                            start=True, stop=True, skip_group_check=True,
                            tile_position=(32 * hi, 64 * hj))
                eT = []
                for p2, bank in enumerate((bankA, bankB)):
                    nc.vector.tensor_tensor(
                        out=bank[:, :], in0=bank[:, :],
                        in1=cb_sb[c % 8][:, 2 * hq + p2, :],
                        op=mybir.AluOpType.add)
                    e = epool.tile([128, 8, 64], BF16, tag=f"e{p2}")
                    nc.scalar.activation(
                        e.rearrange("p s m -> p (s m)"), bank[:, :],
                        mybir.ActivationFunctionType.Exp, scale=SCALE)
                    eT.append(e)
                # denominator, partition-replicated via selector matmul
                dbank = pden.tile([128, 512], F32, tag="den")
                for p2 in range(2):
                    nc.tensor.matmul(
                        dbank[64 * p2:64 * (p2 + 1), :],
                        sel_sb[:, :],
                        eT[p2].rearrange("p s m -> p (s m)"),
                        start=True, stop=True, skip_group_check=True,
                        tile_position=(0, 64 * p2))
                rden = small.tile([128, 512], F32, tag="rden")
                nc.vector.reciprocal(rden[:, :], dbank[:, :])
                # PV: v stationary, e moving -> obank [32hi+d, 64wi+n]
                obank = pout.tile([128, 512], F32, tag="pot")
                for wi in range(8):
                    for hi in range(4):
                        hj = hi % 2
                        h = 4 * hq + hi
                        nc.tensor.matmul(
                            obank[32 * hi:32 * (hi + 1), 64 * wi:64 * (wi + 1)],
                            s["vd"][64 * hj:64 * (hj + 1), wi, HD * h:HD * (h + 1)],
                            eT[hi // 2][64 * hj:64 * (hj + 1), wi, :],
                            start=True, stop=True, skip_group_check=True,
                            tile_position=(64 * hj, 32 * hi))
                # stage to ot (partition 32*(h%4)+d, free (t, q, w2, m)),
                # normalizing by rden on the way out
                dst = s["ot"].rearrange("p (t q w m) -> p t q w m", t=4, q=4, w=2) \
                    [:, :, hq, :, :]
                nc.vector.tensor_tensor(
                    out=dst,
                    in0=obank.rearrange("p (t w m) -> p t w m", t=4, w=2),
                    in1=rden.rearrange("p (t w m) -> p t w m", t=4, w=2),
                    op=mybir.AluOpType.mult)

            def emit_proj_group(c, rt):
                s = st[c]
                r0 = c * ROWS_PER_CHUNK
                bank = pbig.tile([128, C], F32, tag="pq")
                ot4 = s["ot"].rearrange("p (t q f) -> p t q f", t=4, q=4)
                for quad in range(4):
                    nc.tensor.matmul(
                        bank[:, :],
                        ot4[:, rt, quad, :],
                        wp_sb[:, quad, :],
                        start=(quad == 0), stop=(quad == 3))
                out_f32 = small.tile([128, C], F32, tag="outf")
                nc.vector.tensor_tensor(out=out_f32[:, :], in0=bank[:, :],
                                        in1=bp_sb[:, :], op=mybir.AluOpType.add)
                nc.sync.dma_start(
                    out=out[r0 + 128 * rt:r0 + 128 * (rt + 1), :],
                    in_=out_f32[:, :])

            # software pipeline: big qkv groups of chunk c interleaved with
            # attention/proj groups of chunk c-1
            for c in range(n_chunks + 1):
                if c < n_chunks:
                    emit_dma(c)
                big = [("qkv", c, g) for g in range(12)] if c < n_chunks else []
                smalls = ([("attn", c - 1, hq) for hq in range(4)]
                          + [("proj", c - 1, rt) for rt in range(4)]) if c > 0 else []
                order = []
                for i in range(max(len(big), len(smalls))):
                    if i < len(big):
                        order.append(big[i])
                    if i < len(smalls):
                        order.append(smalls[i])
                for kind, cc, idx in order:
                    if kind == "qkv":
                        emit_qkv_group(cc, idx)
                    elif kind == "attn":
                        emit_attn_group(cc, idx)
                    else:
                        emit_proj_group(cc, idx)
                if c > 0:
                    del st[c - 1]
    nc.compile()
    return nc


_NC_CACHE = {}


def _get_nc(n_chunks):
    if n_chunks not in _NC_CACHE:
        _NC_CACHE[n_chunks] = build_nc(n_chunks)
    return _NC_CACHE[n_chunks]


def prep_shared(w_qkv1, b_qkv1, w_qkv2, b_qkv2, bias_table, rel_index, w_proj,
                b_proj, mask):
    """Host-side prep of weights/bias tables shared by all cores."""
    # q/k weight columns in fp8 (pre-scaled), laid out [128, kt, cols]
    w1qk = (w_qkv1[:, :2 * C] * WS).reshape(KT_X, 128, 2 * C)
    w1qk = np.ascontiguousarray(w1qk.transpose(1, 0, 2)).astype(E4)
    w2kf = np.zeros((CYP, C), np.float32)
    w2kf[:CY] = w_qkv2[:, C:2 * C] * WS
    w2kf[CY] = (b_qkv1[C:2 * C] + b_qkv2[C:2 * C]) * WS  # k bias on ones-row
    w2k = np.ascontiguousarray(
        w2kf.reshape(KT_Y, 128, C).transpose(1, 0, 2)).astype(E4)
    # v weight columns in bf16 (unscaled)
    w1v = np.ascontiguousarray(
        w_qkv1[:, 2 * C:].reshape(KT_X, 128, C).transpose(1, 0, 2)).astype(BF)
    w2vf = np.zeros((CYP, C), np.float32)
    w2vf[:CY] = w_qkv2[:, 2 * C:]
    w2vf[CY] = b_qkv1[2 * C:] + b_qkv2[2 * C:]           # v bias on ones-row
    w2v = np.ascontiguousarray(
        w2vf.reshape(KT_Y, 128, C).transpose(1, 0, 2)).astype(BF)

    wp = np.ascontiguousarray(w_proj.reshape(4, 128, C).transpose(1, 0, 2)) \
        .astype(BF)

    bq = b_qkv1[0:C].reshape(FT_Q, 128).T.astype(np.float32).copy()
    bp = np.broadcast_to(b_proj.astype(np.float32), (128, C)).copy()

    # combined bias table, TRANSPOSED blocks ([m, n]) in the logits-bank
    # layout: bank = 2*hq + p2; partition 64*hj + m; free 64*wi + n
    rpb = bias_table[rel_index.reshape(-1)].reshape(N, N, H).transpose(2, 0, 1)
    cbt = (rpb[None] + mask[:, None]) / SCALE            # [w, h, n, m] f32
    cbtT = cbt.transpose(0, 1, 3, 2)                     # [w, h, m, n]
    cbd = np.empty((8, 8, 128, 512), np.float32)
    for c8 in range(8):
        for hq in range(4):
            for p2 in range(2):
                for hj in range(2):
                    for wi in range(8):
                        w = 8 * c8 + wi
                        h = 4 * hq + 2 * p2 + hj
                        cbd[c8, 2 * hq + p2, 64 * hj:64 * hj + 64,
                            64 * wi:64 * wi + 64] = cbtT[w, h]
    cbd = np.ascontiguousarray(cbd.transpose(0, 2, 1, 3)).astype(BF)

    # selector: sel[k, p] = 1 iff p//32 == k//64  (sums partition halves)
    kk = np.arange(128)[:, None]
    pp = np.arange(64)[None, :]
    sel = (pp // 32 == kk // 64).astype(np.float32).astype(BF)

    return dict(w1qk=w1qk, w2k=w2k, w1v=w1v, w2v=w2v, wp=wp, bq=bq, bp=bp,
                cb=cbd, sel=sel)


def _chunk_major(a_t, n_chunks, kt):
    """[feat, rows] -> [n_chunks, 128, kt, 512] chunk-major layout."""
    return np.ascontiguousarray(
        a_t.reshape(kt, 128, n_chunks, 512).transpose(2, 1, 0, 3))


def prep_core_inputs(x, y, shared, n_cores=N_CORES):
    """Split x, y along batch, transpose to feature-major, fp8+bf16."""
    B_, n, _ = x.shape
    rows = (B_ // n_cores) * n
    n_chunks = rows // ROWS_PER_CHUNK
    in_maps = []
    for i in range(n_cores):
        lo = i * (B_ // n_cores)
        hi = lo + B_ // n_cores
        xs = x[lo:hi].reshape(rows, CX)
        ys = y[lo:hi].reshape(rows, CY)
        xT = np.ascontiguousarray(xs.T)
        yT = np.zeros((CYP, rows), np.float32)
        yT[:CY] = ys.T
        yT[CY] = 1.0
        in_maps.append(dict(
            xt8=_chunk_major(xT.astype(E4), n_chunks, KT_X),
            yt8=_chunk_major(yT.astype(E4), n_chunks, KT_Y),
            xb=_chunk_major(xT.astype(BF), n_chunks, KT_X),
            yb=_chunk_major(yT.astype(BF), n_chunks, KT_Y),
            **shared))
    return in_maps


def kernel(x, y, mask, w_qkv1, b_qkv1, w_qkv2, b_qkv2, bias_table, rel_index,
           w_proj, b_proj, _n_cores=N_CORES, _trace=False):
    B_, n, _ = x.shape
    n_chunks = (B_ // _n_cores) // WIN_PER_CHUNK
    shared = prep_shared(np.asarray(w_qkv1), np.asarray(b_qkv1),
                         np.asarray(w_qkv2), np.asarray(b_qkv2),
                         np.asarray(bias_table), np.asarray(rel_index),
                         np.asarray(w_proj), np.asarray(b_proj),
                         np.asarray(mask))
    in_maps = prep_core_inputs(np.asarray(x), np.asarray(y), shared, _n_cores)
    nc = _get_nc(n_chunks)
    res = run_bass_kernel_spmd(nc, in_maps, core_ids=list(range(_n_cores)))
    outs = [res.results[i]["out"].reshape(B_ // _n_cores, n, C)
            for i in range(_n_cores)]
    full = np.concatenate(outs, axis=0)
    kernel.last_results = res
    kernel.last_ctx = (nc, in_maps)
    return full


# revision 11
# speedup vs baseline: 1.1344x; 1.0513x over previous
"""Cross-WindowAttention Trainium2 kernel (v3).

Full inputs -> shard batch dim over 8 NeuronCores -> bass/Tile kernel per core
-> gather. Host-side numpy does layout prep (chunk-major feature-major
transposes, fp8/bf16 conversion, combined rpb+mask bias table); the Bass
kernel does all matmul/softmax compute.

Per-core pipeline (shard = 256 windows of 64 tokens, 16384 rows):
 - q/k projections on PE in fp8e4 with DoubleRow perf mode (weights
   pre-scaled x16 on host, de-scaled at the PSUM->SBUF copy). v projection
   in bf16 (fp8 v fails the accuracy gate); v rows are duplicated into both
   partition halves (small DMAs) so PV matmuls can pack.
 - attention logits computed TRANSPOSED ([m, n]; k stationary) per
   (hq = 4 heads) x (8 windows) group into two PSUM banks, each holding a
   diagonal pair of PE tile positions; the two banks' matmuls overlap on
   the array (verified: same-column-group tiles are only legal across
   different PSUM banks).  Softmax denominator via a host-constant selector
   matmul that lands partition-replicated [128, 512]; normalization is
   fused into the PV->SBUF staging multiply on DVE.  No PE transposes.
 - output projection with attention-output tiles stationary in bf16,
   bias via DVE add -> row-major result, contiguous DMA out.

The chunk loop is software-pipelined by one chunk: attention/proj groups of
chunk c-1 are emitted interleaved between the qkv matmul groups of chunk c.
"""
import numpy as np
import ml_dtypes

import concourse.bacc as bacc
import concourse.mybir as mybir
from concourse.tile import TileContext
from concourse.bass_utils import run_bass_kernel_spmd

F32 = mybir.dt.float32
BF16 = mybir.dt.bfloat16
FP8 = mybir.dt.float8e4
BF = ml_dtypes.bfloat16
E4 = ml_dtypes.float8_e4m3

N_CORES = 8
B_FULL = 2048
N = 64                      # window size (tokens per window)
C = 512                     # channels
H = 16                      # heads
HD = 32                     # head dim
CX = 512                    # x feature dim
CY = 1000                   # y feature dim
CYP = 1024                  # y feature dim padded to multiple of 128
SCALE = HD ** -0.5
WS = 16.0                   # fp8 weight pre-scale
INV_WS = 1.0 / WS

B_SHARD = B_FULL // N_CORES             # 256 windows per core
WIN_PER_CHUNK = 8
ROWS_PER_CHUNK = WIN_PER_CHUNK * N      # 512
N_CHUNKS = B_SHARD // WIN_PER_CHUNK     # 32

KT_X = CX // 128            # 4 contraction tiles from x
KT_Y = CYP // 128            # 8 contraction tiles from y (padded)
FT_Q = C // 128             # 4 feature tiles per projection output

DR = mybir.MatmulPerfMode.DoubleRow


def build_nc(n_chunks=N_CHUNKS, ablate=frozenset()):
    rows = n_chunks * ROWS_PER_CHUNK
    nc = bacc.Bacc("TRN2", target_bir_lowering=False)

    xt8 = nc.dram_tensor("xt8", [n_chunks, 128, KT_X, 512], FP8, kind="ExternalInput")
    yt8 = nc.dram_tensor("yt8", [n_chunks, 128, KT_Y, 512], FP8, kind="ExternalInput")
    xb = nc.dram_tensor("xb", [n_chunks, 128, KT_X, 512], BF16, kind="ExternalInput")
    yb = nc.dram_tensor("yb", [n_chunks, 128, KT_Y, 512], BF16, kind="ExternalInput")
    w1qk = nc.dram_tensor("w1qk", [128, KT_X, 2 * C], FP8, kind="ExternalInput")
    w2k = nc.dram_tensor("w2k", [128, KT_Y, C], FP8, kind="ExternalInput")
    w1v = nc.dram_tensor("w1v", [128, KT_X, C], BF16, kind="ExternalInput")
    w2v = nc.dram_tensor("w2v", [128, KT_Y, C], BF16, kind="ExternalInput")
    wp = nc.dram_tensor("wp", [128, 4, C], BF16, kind="ExternalInput")
    cb = nc.dram_tensor("cb", [8, 128, 8, 512], BF16, kind="ExternalInput")
    bq = nc.dram_tensor("bq", [128, FT_Q], F32, kind="ExternalInput")
    bp = nc.dram_tensor("bp", [128, C], F32, kind="ExternalInput")
    sel = nc.dram_tensor("sel", [128, 64], BF16, kind="ExternalInput")
    ones32 = nc.dram_tensor("ones32", [128, 32], BF16, kind="ExternalInput")
    out = nc.dram_tensor("out", [rows, C], F32, kind="ExternalOutput")

    with TileContext(nc) as tc:
        with tc.tile_pool(name="const", bufs=1) as constp, \
             tc.tile_pool(name="wpool", bufs=1) as wpool, \
             tc.tile_pool(name="stream", bufs=2) as stream, \
             tc.tile_pool(name="acts", bufs=2) as acts, \
             tc.tile_pool(name="epool", bufs=2) as epool, \
             tc.tile_pool(name="small", bufs=3) as small, \
             tc.tile_pool(name="pbig", bufs=2, space="PSUM") as pbig, \
             tc.tile_pool(name="ploga", bufs=2, space="PSUM") as ploga, \
             tc.tile_pool(name="plogb", bufs=2, space="PSUM") as plogb, \
             tc.tile_pool(name="pden", bufs=1, space="PSUM") as pden, \
             tc.tile_pool(name="pout", bufs=1, space="PSUM") as pout:

            # ---- resident constants / weights
            w1qk_sb = wpool.tile([128, KT_X, 2 * C], FP8)
            nc.sync.dma_start(out=w1qk_sb, in_=w1qk[:, :, :])
            w2k_sb = wpool.tile([128, KT_Y, C], FP8)
            nc.sync.dma_start(out=w2k_sb, in_=w2k[:, :, :])
            w1v_sb = wpool.tile([128, KT_X, C], BF16)
            nc.sync.dma_start(out=w1v_sb, in_=w1v[:, :, :])
            w2v_sb = wpool.tile([128, KT_Y, C], BF16)
            nc.sync.dma_start(out=w2v_sb, in_=w2v[:, :, :])
            wp_sb = wpool.tile([128, 4, C], BF16)
            nc.sync.dma_start(out=wp_sb, in_=wp[:, :, :])
            bq_sb = constp.tile([128, FT_Q], F32)
            nc.sync.dma_start(out=bq_sb, in_=bq[:, :])
            bp_sb = constp.tile([128, C], F32)
            nc.sync.dma_start(out=bp_sb, in_=bp[:, :])
            sel_sb = constp.tile([128, 64], BF16)
            nc.sync.dma_start(out=sel_sb, in_=sel[:, :])
            ones_sb = constp.tile([128, 32], BF16)
            nc.sync.dma_start(out=ones_sb, in_=ones32[:, :])
            cb_sb = []
            for c8 in range(min(8, n_chunks)):
                t = constp.tile([128, 8, 512], BF16, name=f"cb{c8}")
                nc.sync.dma_start(out=t, in_=cb[c8])
                cb_sb.append(t)

            st = {}  # per-chunk live tiles

            def emit_dma(c):
                s = {}
                s["xt8"] = stream.tile([128, KT_X, 512], FP8, tag="xt8", name="xt8")
                nc.sync.dma_start(out=s["xt8"], in_=xt8[c])
                s["yt8"] = stream.tile([128, KT_Y, 512], FP8, tag="yt8", name="yt8")
                nc.sync.dma_start(out=s["yt8"], in_=yt8[c])
                s["xb"] = stream.tile([128, KT_X, 512], BF16, tag="xb", name="xb")
                nc.sync.dma_start(out=s["xb"], in_=xb[c])
                s["yb"] = stream.tile([128, KT_Y, 512], BF16, tag="yb", name="yb")
                nc.sync.dma_start(out=s["yb"], in_=yb[c])
                s["q"] = acts.tile([128, FT_Q, 512], BF16, tag="q", name="qsb")
                s["k"] = acts.tile([128, FT_Q, 512], BF16, tag="k", name="ksb")
                s["vd"] = acts.tile([128, 8, C], BF16, tag="vd", name="vdsb")
                s["ot"] = acts.tile([128, 4 * 512], BF16, tag="ot", name="otsb")
                st[c] = s

            def emit_qkv_group(c, g):
                s = st[c]
                if g < FT_Q:                      # q projection, feature tile g
                    ft = g
                    bank = pbig.tile([128, 512], F32, tag="pq")
                    if "nodr" in ablate:
                        for kt in range(KT_X):
                            nc.tensor.matmul(
                                bank[:, :],
                                w1v_sb[:, kt, 128 * (ft % 4):128 * (ft % 4) + 128]
                                if False else w1v_sb[:, kt, 0:128],
                                s["xb"][:, kt, :],
                                start=(kt == 0), stop=(kt == KT_X - 1))
                    else:
                        for kp in range(KT_X // 2):
                            nc.tensor.matmul(
                                bank[:, :],
                                w1qk_sb[:, 2 * kp:2 * kp + 2, 128 * ft:128 * (ft + 1)],
                                s["xt8"][:, 2 * kp:2 * kp + 2, :],
                                start=(kp == 0), stop=(kp == KT_X // 2 - 1),
                                perf_mode=DR)
                    if "nocopy" not in ablate:
                        nc.scalar.activation(
                            s["q"][:, ft, :], bank[:, :],
                            mybir.ActivationFunctionType.Identity,
                            bias=bq_sb[:, ft:ft + 1], scale=INV_WS)
                elif g < 2 * FT_Q:                # k projection, feature tile g-4
                    ft = g - FT_Q
                    bank = pbig.tile([128, 512], F32, tag="pq")
                    if "nodr" in ablate:
                        for kt in range(KT_X):
                            nc.tensor.matmul(
                                bank[:, :],
                                w1v_sb[:, kt, 0:128],
                                s["xb"][:, kt, :],
                                start=(kt == 0), stop=False)
                        for kt in range(KT_Y):
                            nc.tensor.matmul(
                                bank[:, :],
                                w2v_sb[:, kt, 0:128],
                                s["yb"][:, kt, :],
                                start=False, stop=(kt == KT_Y - 1))
                    else:
                        for kp in range(KT_X // 2):
                            nc.tensor.matmul(
                                bank[:, :],
                                w1qk_sb[:, 2 * kp:2 * kp + 2,
                                        C + 128 * ft:C + 128 * (ft + 1)],
                                s["xt8"][:, 2 * kp:2 * kp + 2, :],
                                start=(kp == 0), stop=False, perf_mode=DR)
                        for kp in range(KT_Y // 2):
                            nc.tensor.matmul(
                                bank[:, :],
                                w2k_sb[:, 2 * kp:2 * kp + 2, 128 * ft:128 * (ft + 1)],
                                s["yt8"][:, 2 * kp:2 * kp + 2, :],
                                start=False, stop=(kp == KT_Y // 2 - 1), perf_mode=DR)
                    if "nocopy" not in ablate:
                        if "actcopy" in ablate:
                            nc.scalar.mul(s["k"][:, ft, :], bank[:, :], INV_WS)
                        else:
                            nc.vector.tensor_scalar_mul(s["k"][:, ft, :],
                                                        bank[:, :], INV_WS)
                else:                             # v projection, row tile g-8
                    rt = g - 2 * FT_Q
                    bank = pbig.tile([128, C], F32, tag="pq")
                    for kt in range(KT_X):
                        nc.tensor.matmul(
                            bank[:, :],
                            s["xb"][:, kt, 128 * rt:128 * (rt + 1)],
                            w1v_sb[:, kt, :],
                            start=(kt == 0), stop=False)
                    for kt in range(KT_Y):
                        nc.tensor.matmul(
                            bank[:, :],
                            s["yb"][:, kt, 128 * rt:128 * (rt + 1)],
                            w2v_sb[:, kt, :],
                            start=False, stop=(kt == KT_Y - 1))
                    if "nocopy" not in ablate:
                        # windows (2rt, 2rt+1): duplicate rows into both halves
                        if "actcopy" in ablate:
                            nc.scalar.copy(s["vd"][0:64, 2 * rt, :],
                                           bank[0:64, :])
                            nc.scalar.copy(s["vd"][64:128, 2 * rt + 1, :],
                                           bank[64:128, :])
                        else:
                            nc.vector.tensor_copy(s["vd"][0:64, 2 * rt, :],
                                                  bank[0:64, :])
                            nc.vector.tensor_copy(s["vd"][64:128, 2 * rt + 1, :],
                                                  bank[64:128, :])
                        if "nodup" not in ablate and rt == 3:
                            # one strided DMA per parity for all 4 row tiles
                            vd2 = s["vd"].rearrange("p (a b) f -> p a b f", b=2)
                            nc.sync.dma_start(
                                out=vd2[64:128, :, 0, :],
                                in_=vd2[0:64, :, 0, :])
                            nc.sync.dma_start(
                                out=vd2[0:64, :, 1, :],
                                in_=vd2[64:128, :, 1, :])

            def emit_attn_group(c, hq):
                if "noattn" in ablate:
                    return
                s = st[c]
                bankA = ploga.tile([128, 512], F32, tag="la")
                bankB = plogb.tile([128, 512], F32, tag="lb")
                # QK^T transposed: k stationary -> logits block [m, n]
                for wi in range(8):
                    for hi in range(4):
                        bank = bankA if hi < 2 else bankB
                        hj = hi % 2
                        nc.tensor.matmul(
                            bank[64 * hj:64 * (hj + 1), 64 * wi:64 * (wi + 1)],
                            s["k"][32 * hi:32 * hi + 32, hq, 64 * wi:64 * (wi + 1)],
                            s["q"][32 * hi:32 * hi + 32, hq, 64 * wi:64 * (wi + 1)],
                            start=True, stop=True, skip_group_check=True,
                            tile_position=(32 * hi, 64 * hj))
                eT = []
                for p2, bank in enumerate((bankA, bankB)):
                    e = epool.tile([128, 8, 64], BF16, tag=f"e{p2}")
                    ef = e.rearrange("p s m -> p (s m)")
                    nc.scalar.activation(
                        ef, bank[:, :],
                        mybir.ActivationFunctionType.Exp, scale=SCALE)
                    nc.vector.tensor_tensor(
                        out=ef, in0=ef,
                        in1=cb_sb[c % 8][:, 2 * hq + p2, :],
                        op=mybir.AluOpType.mult)
                    eT.append(e)
                # PV (v stationary) + per-block ones-matmul denominators,
                # interleaved; den lands partition-replicated in dbank with
                # the same block layout as obank
                rden = None
                dbank = None
                if "noden" not in ablate:
                    dbank = pden.tile([128, 512], F32, tag="den")
                obank = pout.tile([128, 512], F32, tag="pot")
                for wi in range(8):
                    for hi in range(4):
                        hj = hi % 2
                        h = 4 * hq + hi
                        e_blk = eT[hi // 2][64 * hj:64 * (hj + 1), wi, :]
                        nc.tensor.matmul(
                            obank[32 * hi:32 * (hi + 1), 64 * wi:64 * (wi + 1)],
                            s["vd"][64 * hj:64 * (hj + 1), wi, HD * h:HD * (h + 1)],
                            e_blk,
                            start=True, stop=True, skip_group_check=True,
                            tile_position=(64 * hj, 32 * hi))
                        if dbank is not None:
                            nc.tensor.matmul(
                                dbank[32 * hi:32 * (hi + 1),
                                      64 * wi:64 * (wi + 1)],
                                ones_sb[64 * hj:64 * (hj + 1), :],
                                e_blk,
                                start=True, stop=True, skip_group_check=True,
                                tile_position=(64 * hj, 32 * hi))
                if dbank is not None:
                    rden = small.tile([128, 512], F32, tag="rden")
                    nc.vector.reciprocal(rden[:, :], dbank[:, :])
                # stage to ot (partition 32*(h%4)+d, free (t, q, w2, m)),
                # normalizing by rden on the way out
                dst = s["ot"].rearrange("p (t q w m) -> p t q w m", t=4, q=4, w=2) \
                    [:, :, hq, :, :]
                if rden is None:
                    nc.vector.tensor_copy(
                        dst, obank.rearrange("p (t w m) -> p t w m", t=4, w=2))
                else:
                    nc.vector.tensor_tensor(
                        out=dst,
                        in0=obank.rearrange("p (t w m) -> p t w m", t=4, w=2),
                        in1=rden.rearrange("p (t w m) -> p t w m", t=4, w=2),
                        op=mybir.AluOpType.mult)

            def emit_proj_group(c, rt):
                s = st[c]
                r0 = c * ROWS_PER_CHUNK
                if "noattn" in ablate:
                    out_f32 = small.tile([128, C], F32, tag="outf")
                    nc.vector.memset(out_f32[:, :], 0.0)
                    nc.sync.dma_start(
                        out=out[r0 + 128 * rt:r0 + 128 * (rt + 1), :],
                        in_=out_f32[:, :])
                    return
                bank = pbig.tile([128, C], F32, tag="pq")
                ot4 = s["ot"].rearrange("p (t q f) -> p t q f", t=4, q=4)
                for quad in range(4):
                    nc.tensor.matmul(
                        bank[:, :],
                        ot4[:, rt, quad, :],
                        wp_sb[:, quad, :],
                        start=(quad == 0), stop=(quad == 3))
                out_f32 = small.tile([128, C], F32, tag="outf")
                nc.vector.tensor_tensor(out=out_f32[:, :], in0=bank[:, :],
                                        in1=bp_sb[:, :], op=mybir.AluOpType.add)
                nc.sync.dma_start(
                    out=out[r0 + 128 * rt:r0 + 128 * (rt + 1), :],
                    in_=out_f32[:, :])

            # software pipeline: big qkv groups of chunk c interleaved with
            # attention/proj groups of chunk c-1
            for c in range(n_chunks + 1):
                if c < n_chunks:
                    emit_dma(c)
                big = [("qkv", c, g) for g in range(12)] if c < n_chunks else []
                smalls = ([("attn", c - 1, hq) for hq in range(4)]
                          + [("proj", c - 1, rt) for rt in range(4)]) if c > 0 else []
                order = []
                for i in range(max(len(big), len(smalls))):
                    if i < len(big):
                        order.append(big[i])
                    if i < len(smalls):
                        order.append(smalls[i])
                for kind, cc, idx in order:
                    if kind == "qkv":
                        emit_qkv_group(cc, idx)
                    elif kind == "attn":
                        emit_attn_group(cc, idx)
                    else:
                        emit_proj_group(cc, idx)
                if c > 0:
                    del st[c - 1]
    nc.compile()
    return nc


_NC_CACHE = {}


def _get_nc(n_chunks):
    if n_chunks not in _NC_CACHE:
        _NC_CACHE[n_chunks] = build_nc(n_chunks)
    return _NC_CACHE[n_chunks]


def prep_shared(w_qkv1, b_qkv1, w_qkv2, b_qkv2, bias_table, rel_index, w_proj,
                b_proj, mask):
    """Host-side prep of weights/bias tables shared by all cores."""
    # q/k weight columns in fp8 (pre-scaled), laid out [128, kt, cols]
    w1qk = (w_qkv1[:, :2 * C] * WS).reshape(KT_X, 128, 2 * C)
    w1qk = np.ascontiguousarray(w1qk.transpose(1, 0, 2)).astype(E4)
    w2kf = np.zeros((CYP, C), np.float32)
    w2kf[:CY] = w_qkv2[:, C:2 * C] * WS
    w2kf[CY] = (b_qkv1[C:2 * C] + b_qkv2[C:2 * C]) * WS  # k bias on ones-row
    w2k = np.ascontiguousarray(
        w2kf.reshape(KT_Y, 128, C).transpose(1, 0, 2)).astype(E4)
    # v weight columns in bf16 (unscaled)
    w1v = np.ascontiguousarray(
        w_qkv1[:, 2 * C:].reshape(KT_X, 128, C).transpose(1, 0, 2)).astype(BF)
    w2vf = np.zeros((CYP, C), np.float32)
    w2vf[:CY] = w_qkv2[:, 2 * C:]
    w2vf[CY] = b_qkv1[2 * C:] + b_qkv2[2 * C:]           # v bias on ones-row
    w2v = np.ascontiguousarray(
        w2vf.reshape(KT_Y, 128, C).transpose(1, 0, 2)).astype(BF)

    wp = np.ascontiguousarray(w_proj.reshape(4, 128, C).transpose(1, 0, 2)) \
        .astype(BF)

    bq = b_qkv1[0:C].reshape(FT_Q, 128).T.astype(np.float32).copy()
    bp = np.broadcast_to(b_proj.astype(np.float32), (128, C)).copy()

    # combined bias table, TRANSPOSED blocks ([m, n]) in the logits-bank
    # layout: bank = 2*hq + p2; partition 64*hj + m; free 64*wi + n
    rpb = bias_table[rel_index.reshape(-1)].reshape(N, N, H).transpose(2, 0, 1)
    cbt = np.exp(rpb[None] + mask[:, None])              # [w, h, n, m] f32
    cbtT = cbt.transpose(0, 1, 3, 2)                     # [w, h, m, n]
    cbd = np.empty((8, 8, 128, 512), np.float32)
    for c8 in range(8):
        for hq in range(4):
            for p2 in range(2):
                for hj in range(2):
                    for wi in range(8):
                        w = 8 * c8 + wi
                        h = 4 * hq + 2 * p2 + hj
                        cbd[c8, 2 * hq + p2, 64 * hj:64 * hj + 64,
                            64 * wi:64 * wi + 64] = cbtT[w, h]
    cbd = np.ascontiguousarray(cbd.transpose(0, 2, 1, 3)).astype(BF)

    # selector: sel[k, p] = 1 iff p//32 == k//64  (sums partition halves)
    kk = np.arange(128)[:, None]
    pp = np.arange(64)[None, :]
    sel = (pp // 32 == kk // 64).astype(np.float32).astype(BF)

    ones32 = np.ones((128, 32), np.float32).astype(BF)
    return dict(w1qk=w1qk, w2k=w2k, w1v=w1v, w2v=w2v, wp=wp, bq=bq, bp=bp,
                cb=cbd, sel=sel, ones32=ones32)


def _chunk_major(a_t, n_chunks, kt):
    """[feat, rows] -> [n_chunks, 128, kt, 512] chunk-major layout."""
    return np.ascontiguousarray(
        a_t.reshape(kt, 128, n_chunks, 512).transpose(2, 1, 0, 3))


def prep_core_inputs(x, y, shared, n_cores=N_CORES):
    """Split x, y along batch, transpose to feature-major, fp8+bf16."""
    B_, n, _ = x.shape
    rows = (B_ // n_cores) * n
    n_chunks = rows // ROWS_PER_CHUNK
    in_maps = []
    for i in range(n_cores):
        lo = i * (B_ // n_cores)
        hi = lo + B_ // n_cores
        xs = x[lo:hi].reshape(rows, CX)
        ys = y[lo:hi].reshape(rows, CY)
        xT = np.ascontiguousarray(xs.T)
        yT = np.zeros((CYP, rows), np.float32)
        yT[:CY] = ys.T
        yT[CY] = 1.0
        in_maps.append(dict(
            xt8=_chunk_major(xT.astype(E4), n_chunks, KT_X),
            yt8=_chunk_major(yT.astype(E4), n_chunks, KT_Y),
            xb=_chunk_major(xT.astype(BF), n_chunks, KT_X),
            yb=_chunk_major(yT.astype(BF), n_chunks, KT_Y),
            **shared))
    return in_maps


def kernel(x, y, mask, w_qkv1, b_qkv1, w_qkv2, b_qkv2, bias_table, rel_index,
           w_proj, b_proj, _n_cores=N_CORES, _trace=False):
    B_, n, _ = x.shape
    n_chunks = (B_ // _n_cores) // WIN_PER_CHUNK
    shared = prep_shared(np.asarray(w_qkv1), np.asarray(b_qkv1),
                         np.asarray(w_qkv2), np.asarray(b_qkv2),
                         np.asarray(bias_table), np.asarray(rel_index),
                         np.asarray(w_proj), np.asarray(b_proj),
                         np.asarray(mask))
    in_maps = prep_core_inputs(np.asarray(x), np.asarray(y), shared, _n_cores)
    nc = _get_nc(n_chunks)
    res = run_bass_kernel_spmd(nc, in_maps, core_ids=list(range(_n_cores)))
    outs = [res.results[i]["out"].reshape(B_ // _n_cores, n, C)
            for i in range(_n_cores)]
    full = np.concatenate(outs, axis=0)
    kernel.last_results = res
    kernel.last_ctx = (nc, in_maps)
    return full
